# revision 3
# baseline (speedup 1.0000x reference)
"""Trainium2 Bass kernel for the per-cell-MLP "MAR one-sided missingness" model.

Model (per cell (n,t) of a 1024x128 grid):
    xc     = X[n, col_idx[n,t]]
    h      = relu(W_in[n,t,:,0]*xc + W_in[n,t,:,1]*X[n,t] + b_in[n,t,:])   # [H]
    out    = sigmoid(dot(W_out[n,t,:], h) + b_out[n,t])

Sharding: rows N split across 8 cores (128 rows each), fully data parallel.

Per-core layout: partition dim = t (128 cells of one row), free dim = h.
Per row n the kernel does:
  PE   : cb   = broadcast(col_idx[n,:]) via rank-1 matmul          (PSUM)
  DVE  : oh   = is_equal(cb, iota_j)      -> one-hot [j, t]        (SBUF)
  PE   : xc   = oh^T @ X^T[:, n]          -> gathered X[n, c]      (PSUM col)
  ACT  : a0   = w0 * xc                   (copy with per-part scale)
  DVE  : v    = (w1 * x) + b              (fused scalar_tensor_tensor)
  Pool : u    = a0 + v
  DVE  : (u max 0) * wo, accum_out -> red[:, n]   (fused relu+mul+reduce)
Epilogue: out = sigmoid(red + b_out^T), DMA out, host transposes back.

Weights stream from HBM as one contiguous wall[t, n, kind, h] tensor
(4 MB per 16-row superblock DMA) - the kernel is HBM-bandwidth bound.
"""

import numpy as np

N, T, H = 1024, 128, 128
M = 8            # cores
NR = N // M      # rows per core
G = 16           # rows per superblock (one weight DMA)
S = NR // G

_cache = {}


def _build():
    if "nc" in _cache:
        return _cache["nc"]
    import concourse.bacc as bacc
    import concourse.mybir as mybir
    import concourse.tile as tile

    f32 = mybir.dt.float32
    Alu = mybir.AluOpType
    Act = mybir.ActivationFunctionType

    nc = bacc.Bacc()
    wall = nc.declare_dram_parameter("wall", [T, NR, 4, H], f32, isOutput=False)
    xt = nc.declare_dram_parameter("xt", [T, NR], f32, isOutput=False)
    cidx = nc.declare_dram_parameter("cidx", [1, NR * T], f32, isOutput=False)
    ones1 = nc.declare_dram_parameter("ones1", [1, T], f32, isOutput=False)
    iota = nc.declare_dram_parameter("iota", [128, 1], f32, isOutput=False)
    bout = nc.declare_dram_parameter("bout", [T, NR], f32, isOutput=False)
    out = nc.declare_dram_parameter("out", [T, NR], f32, isOutput=True)

    with tile.TileContext(nc) as tc:
        with (
            tc.tile_pool(name="const", bufs=1) as constp,
            tc.tile_pool(name="wpool", bufs=2) as wpool,
            tc.tile_pool(name="gath", bufs=3) as gathp,
            tc.tile_pool(name="work", bufs=3) as workp,
            tc.tile_pool(name="acc", bufs=1) as accp,
            tc.tile_pool(name="ps", bufs=3, space="PSUM") as psp,
            tc.tile_pool(name="psxc", bufs=2, space="PSUM") as psxcp,
        ):
            xt_sb = constp.tile([T, NR], f32)
            nc.sync.dma_start(xt_sb[:], xt[:])
            ci_sb = constp.tile([1, NR * T], f32)
            nc.sync.dma_start(ci_sb[:], cidx[:])
            on_sb = constp.tile([1, T], f32)
            nc.sync.dma_start(on_sb[:], ones1[:])
            io_sb = constp.tile([128, 1], f32)
            nc.sync.dma_start(io_sb[:], iota[:])
            bo_sb = constp.tile([T, NR], f32)
            nc.sync.dma_start(bo_sb[:], bout[:])

            red = accp.tile([T, NR], f32)

            for s in range(S):
                wt = wpool.tile([128, G * 4 * H], f32, tag="wt")
                nc.sync.dma_start(wt[:], wall[:, s * G : (s + 1) * G])

                xc_ps = psxcp.tile([128, G], f32, tag="xc")
                for g in range(G):
                    n = s * G + g
                    cb = psp.tile([128, T], f32, tag="cb")
                    nc.tensor.matmul(
                        cb[:], on_sb[:], ci_sb[0:1, n * T : (n + 1) * T], start=True, stop=True
                    )
                    oh = gathp.tile([128, T], f32, tag="oh")
                    nc.vector.tensor_scalar(
                        oh[:], cb[:], io_sb[:, 0:1], None, Alu.is_equal
                    )
                    nc.tensor.matmul(
                        xc_ps[:, g : g + 1],
                        oh[:],
                        xt_sb[:, n : n + 1],
                        start=True,
                        stop=True,
                    )
                xc_sb = workp.tile([128, G], f32, tag="xcsb")
                nc.scalar.copy(xc_sb[:], xc_ps[:])

                for g in range(G):
                    n = s * G + g
                    w0 = wt[:, (4 * g + 0) * H : (4 * g + 1) * H]
                    w1 = wt[:, (4 * g + 1) * H : (4 * g + 2) * H]
                    bb = wt[:, (4 * g + 2) * H : (4 * g + 3) * H]
                    wo = wt[:, (4 * g + 3) * H : (4 * g + 4) * H]

                    a0 = workp.tile([128, H], f32, tag="a0")
                    nc.scalar.activation(
                        a0[:], w0, Act.Copy, scale=xc_sb[:, g : g + 1]
                    )
                    v = workp.tile([128, H], f32, tag="v")
                    nc.vector.scalar_tensor_tensor(
                        v[:], w1, xt_sb[:, n : n + 1], bb, Alu.mult, Alu.add
                    )
                    u = workp.tile([128, H], f32, tag="u")
                    nc.gpsimd.tensor_tensor(u[:], a0[:], v[:], Alu.add)
                    junk = workp.tile([128, H], f32, tag="junk")
                    nc.vector.scalar_tensor_tensor(
                        junk[:],
                        u[:],
                        0.0,
                        wo,
                        Alu.max,
                        Alu.mult,
                        accum_out=red[:, n : n + 1],
                    )

            lg = workp.tile([T, NR], f32, tag="lg")
            nc.vector.tensor_tensor(lg[:], red[:], bo_sb[:], Alu.add)
            ot = workp.tile([T, NR], f32, tag="ot")
            nc.scalar.activation(ot[:], lg[:], Act.Sigmoid)
            nc.sync.dma_start(out[:], ot[:])

    nc.compile()
    _cache["nc"] = nc
    return nc


def make_in_maps(X, W_in, b_in, W_out, b_out, col_idx):
    X = np.asarray(X, dtype=np.float32)
    W_in = np.asarray(W_in, dtype=np.float32)
    b_in = np.asarray(b_in, dtype=np.float32)
    W_out = np.asarray(W_out, dtype=np.float32)
    b_out = np.asarray(b_out, dtype=np.float32)
    col_idx = np.asarray(col_idx)

    iota = np.arange(128, dtype=np.float32).reshape(128, 1)
    ones1 = np.ones((1, T), np.float32)
    in_maps = []
    for c in range(M):
        sl = slice(c * NR, (c + 1) * NR)
        Wc = W_in[sl]  # [NR, T, H, 2]
        wall = np.empty((T, NR, 4, H), dtype=np.float32)
        wall[:, :, 0, :] = Wc[:, :, :, 0].transpose(1, 0, 2)
        wall[:, :, 1, :] = Wc[:, :, :, 1].transpose(1, 0, 2)
        wall[:, :, 2, :] = b_in[sl].transpose(1, 0, 2)
        wall[:, :, 3, :] = W_out[sl].transpose(1, 0, 2)
        in_maps.append(
            {
                "wall": wall,
                "xt": np.ascontiguousarray(X[sl].T),
                "cidx": col_idx[sl].astype(np.float32).reshape(1, -1),
                "ones1": ones1,
                "iota": iota,
                "bout": np.ascontiguousarray(b_out[sl].T),
            }
        )
    return in_maps


def kernel(X, W_in, b_in, W_out, b_out, col_idx):
    from concourse.bass_utils import run_bass_kernel_spmd

    nc = _build()
    in_maps = make_in_maps(X, W_in, b_in, W_out, b_out, col_idx)
    res = run_bass_kernel_spmd(nc, in_maps, list(range(M))).results
    out = np.empty((N, T), np.float32)
    for c in range(M):
        out[c * NR : (c + 1) * NR] = res[c]["out"].T
    return out
